# revision 4
# baseline (speedup 1.0000x reference)
"""Self-contained Trainium2 Bass kernel for a single-layer GRU
(H=2048, L=2048, batch=1), tensor-parallel across 8 NeuronCores.

Strategy: gate-dimension tensor parallelism.  Core d owns a 256-row
slice of each gate (r, z, n) => 768 rows of w_ih / w_hh.  Phase A
computes GIT = w_ih_d @ x.T + bias on-device (batched GEMM, fully
parallel).  Phase B runs the 2048-step recurrence: per step, an
h-stationary matvec streams the resident w_hh_d.T shard through the PE
array, the gh row-vector is transposed to a 128-partition column
layout with PE transposes, gate math runs on DVE/ACT, and the new
256-element h slice is exchanged between all 8 cores with a 1KB
AllGather collective.  All cores execute the same NEFF; per-core
variation is input data only (pre-sharded weights).
"""
import numpy as np

import os

H = 2048
L = 2048
KC = H // 128
NB = 768
NC6 = NB // 128
N_CORES = 8
LC = int(os.environ.get("GRU_CHUNK", "512"))   # steps per NEFF invocation

_cache = {}


def _install_ntff_hook():
    import sys, types
    if "antenv.axon_hooks" in sys.modules:
        return
    try:
        import antenv  # noqa
        import trn_agent_boot.trn_boot as tb
        mod = types.ModuleType("antenv.axon_hooks")
        hook = tb._ntff_profile_via_ctypes('/opt/axon/libaxon_pjrt.so')
        mod.get_axon_ntff_profile_hook = lambda: hook
        mod.set_axon_ntff_profile_hook = lambda h: None
        sys.modules["antenv.axon_hooks"] = mod
    except Exception:
        pass


def _build(Lc=None):
    if Lc is None:
        Lc = LC
    if "nc" in _cache:
        return _cache["nc"]
    import concourse.bacc as bacc
    import concourse.mybir as mybir
    import concourse.tile as tile

    F32 = mybir.dt.float32
    AF = mybir.ActivationFunctionType
    ALU = mybir.AluOpType
    tau = 256

    nc = bacc.Bacc("TRN2", target_bir_lowering=False, num_devices=N_CORES)

    xT = nc.dram_tensor("xT", [H, Lc], F32, kind="ExternalInput")
    h0 = nc.dram_tensor("h0", [128, KC], F32, kind="ExternalInput")
    wih_t = nc.dram_tensor("wih_t", [H, NB], F32, kind="ExternalInput")
    whh_t = nc.dram_tensor("whh_t", [H, NB], F32, kind="ExternalInput")
    bias_gi = nc.dram_tensor("bias_gi", [1, NB], F32, kind="ExternalInput")
    bias_n = nc.dram_tensor("bias_n", [1, 256], F32, kind="ExternalInput")
    h0_own = nc.dram_tensor("h0_own", [128, 2], F32, kind="ExternalInput")
    out = nc.dram_tensor("out", [256, Lc], F32, kind="ExternalOutput")
    h_out = nc.dram_tensor("h_out", [128, KC], F32, kind="ExternalOutput")

    with tile.TileContext(nc) as tc:
        with tc.tile_pool(name="sb", bufs=1) as sb, \
             tc.tile_pool(name="wpool", bufs=3) as wpool, \
             tc.tile_pool(name="xpool", bufs=1) as xpool, \
             tc.tile_pool(name="psA", bufs=1, space="PSUM") as psA, \
             tc.tile_pool(name="psB", bufs=1, space="PSUM") as psB, \
             tc.tile_pool(name="dram", bufs=1, space="DRAM") as dram:

            WT = sb.tile([128, KC * NB], F32)
            GIT = sb.tile([128, NC6 * Lc], F32)
            stage = sb.tile([128, 2 * Lc], F32)
            h_sb = sb.tile([128, KC], F32)
            h_own = sb.tile([128, 2], F32)
            ones = sb.tile([1, 512], F32)
            bgi = sb.tile([1, NB], F32)
            bn = sb.tile([1, 256], F32)
            ghrow = sb.tile([1, NB], F32)
            t_rz = sb.tile([128, 4], F32)
            s_rz = sb.tile([128, 4], F32)
            t1 = sb.tile([128, 2], F32)
            t2 = sb.tile([128, 2], F32)
            nn_t = sb.tile([128, 2], F32)
            dd = sb.tile([128, 2], F32)
            ee = sb.tile([128, 2], F32)

            bounce_in = dram.tile([128, 2], F32)
            bounce_out = dram.tile([N_CORES, 128, 2], F32)

            nc.sync.dma_start(h_sb[:], h0[:])
            # own slice of h0 = columns [2d, 2d+1]; supplied via bias-free
            # trick: host packs own slice into h0 as well -> copy from h_sb
            # is core-dependent, so host supplies it separately via bias_n...
            # simplest: host packs own slice in extra input
            nc.vector.memset(ones[:], 1.0)
            nc.sync.dma_start(h_own[:], h0_own[:])
            nc.sync.dma_start(bgi[:], bias_gi[:])
            nc.sync.dma_start(bn[:], bias_n[:])
            for k in range(KC):
                nc.sync.dma_start(WT[:, k * NB:(k + 1) * NB],
                                  whh_t[128 * k:128 * (k + 1), :])

            # Phase A: GIT = wih_d @ x.T + bias_gi
            for it in range(Lc // tau):
                xt = [xpool.tile([128, tau], F32, tag=f"xt{k}", name=f"xt{k}")
                      for k in range(KC)]
                for k in range(KC):
                    nc.sync.dma_start(xt[k][:], xT[128 * k:128 * (k + 1),
                                                   it * tau:(it + 1) * tau])
                ps = [psA.tile([128, tau], F32, tag=f"psA{c}", name=f"psA{c}")
                      for c in range(NC6)]
                for k in range(KC):
                    w = wpool.tile([128, NB], F32, tag="wih", name="wih")
                    nc.sync.dma_start(w[:], wih_t[128 * k:128 * (k + 1), :])
                    for c in range(NC6):
                        nc.tensor.matmul(ps[c][:], w[:, 128 * c:128 * (c + 1)],
                                         xt[k][:], start=(k == 0), stop=False)
                for c in range(NC6):
                    nc.tensor.matmul(ps[c][:], bgi[0:1, 128 * c:128 * (c + 1)],
                                     ones[0:1, 0:tau], start=False, stop=True)
                    nc.scalar.activation(GIT[:, c * Lc + it * tau:
                                             c * Lc + (it + 1) * tau],
                                         ps[c][:], AF.Copy)

            GIT_r = GIT[:].rearrange("p (c t) -> p c t", c=NC6)

            # Phase B: recurrence
            ps_rz = psB.tile([1, 512], F32)
            ps_n = psB.tile([1, 256], F32)
            ps_t = psA.tile([128, NC6], F32, tag="psA0", name="ps_t")

            for t in range(Lc):
                for k in range(KC):
                    hk = h_sb[:, k:k + 1]
                    nc.tensor.matmul(ps_rz[0:1, :], hk,
                                     WT[:, k * NB:k * NB + 512],
                                     start=(k == 0), stop=(k == KC - 1))
                    nc.tensor.matmul(ps_n[0:1, :], hk,
                                     WT[:, k * NB + 512:(k + 1) * NB],
                                     start=(k == 0), stop=False)
                nc.tensor.matmul(ps_n[0:1, :], ones[0:1, 0:1], bn[0:1, :],
                                 start=False, stop=True)
                nc.scalar.activation(ghrow[0:1, 0:512], ps_rz[:], AF.Copy)
                nc.vector.tensor_copy(ghrow[0:1, 512:NB], ps_n[:])
                for c in range(NC6):
                    nc.tensor.transpose(ps_t[:, c:c + 1],
                                        ghrow[0:1, 128 * c:128 * (c + 1)],
                                        ones[0:1, 0:1])
                git_rz = GIT_r[:, 0:4, t:t + 1].rearrange("p c o -> p (c o)")
                git_n = GIT_r[:, 4:6, t:t + 1].rearrange("p c o -> p (c o)")
                nc.vector.tensor_tensor(t_rz[:], ps_t[:, 0:4], git_rz, ALU.add)
                nc.scalar.activation(s_rz[:], t_rz[:], AF.Sigmoid)
                nc.vector.tensor_tensor(t1[:], s_rz[:, 0:2], ps_t[:, 4:6],
                                        ALU.mult)
                nc.vector.tensor_tensor(t2[:], t1[:], git_n, ALU.add)
                nc.scalar.activation(nn_t[:], t2[:], AF.Tanh)
                nc.vector.tensor_tensor(dd[:], h_own[:], nn_t[:], ALU.subtract)
                nc.vector.tensor_tensor(ee[:], s_rz[:, 2:4], dd[:], ALU.mult)
                h_new = stage[:, 2 * t:2 * t + 2]
                nc.vector.tensor_tensor(h_new, ee[:], nn_t[:], ALU.add)
                nc.vector.tensor_copy(h_own[:], h_new)
                nc.sync.dma_start(bounce_in[:], h_new)
                nc.gpsimd.collective_compute(
                    "AllGather", ALU.bypass,
                    replica_groups=[list(range(N_CORES))],
                    ins=[bounce_in[:]], outs=[bounce_out[:]])
                nc.sync.dma_start(h_sb[:],
                                  bounce_out[:].rearrange("c p j -> p c j"))

            nc.sync.dma_start(h_out[:], h_sb[:])
            stage_v = stage[:].rearrange("p (t j) -> p j t", j=2)
            for j in range(2):
                for q0 in range(0, Lc, 256):
                    qw = min(256, Lc - q0)
                    nc.sync.dma_start(out[128 * j:128 * (j + 1), q0:q0 + qw],
                                      stage_v[:, j:j + 1, q0:q0 + qw])
    nc.compile()
    _cache["nc"] = nc
    return nc


def kernel(x, w_ih, w_hh, b_ih, b_hh):
    _install_ntff_hook()
    from concourse.bass_utils import run_bass_kernel_spmd

    x = np.asarray(x, np.float32)
    w_ih = np.asarray(w_ih, np.float32)
    w_hh = np.asarray(w_hh, np.float32)
    b_ih = np.asarray(b_ih, np.float32)
    b_hh = np.asarray(b_hh, np.float32)

    nc = _build()

    shards = []
    for d in range(N_CORES):
        rows = np.r_[256 * d:256 * (d + 1),
                     H + 256 * d:H + 256 * (d + 1),
                     2 * H + 256 * d:2 * H + 256 * (d + 1)]
        bias_gi = b_ih[rows].copy()
        bias_gi[:512] += b_hh[rows[:512]]
        shards.append({
            "wih_t": np.ascontiguousarray(w_ih[rows].T),
            "whh_t": np.ascontiguousarray(w_hh[rows].T),
            "bias_gi": bias_gi.reshape(1, NB),
            "bias_n": b_hh[rows[512:]].reshape(1, 256).copy(),
        })

    outputs = np.empty((L, H), np.float32)
    h = np.zeros(H, np.float32)
    for c0 in range(0, L, LC):
        xT = np.ascontiguousarray(x[c0:c0 + LC].T)
        h_sb = np.ascontiguousarray(h.reshape(KC, 128).T)  # [128, KC]
        in_maps = []
        for d in range(N_CORES):
            m = dict(shards[d])
            m["xT"] = xT
            m["h0"] = h_sb
            m["h0_own"] = np.ascontiguousarray(
                h[256 * d:256 * (d + 1)].reshape(2, 128).T)
            in_maps.append(m)
        res = run_bass_kernel_spmd(nc, in_maps, core_ids=list(range(N_CORES)))
        for d in range(N_CORES):
            outputs[c0:c0 + LC, 256 * d:256 * (d + 1)] = res.results[d]["out"].T
        h = np.ascontiguousarray(res.results[0]["h_out"].T.reshape(H))
    hidden = outputs[-1].reshape(1, 1, H).copy()
    return outputs, hidden, hidden
